# revision 1
# baseline (speedup 1.0000x reference)
"""Trainium2 Bass kernel for nn_Conv2dLocal (locally-connected 2d conv,
no weight sharing).

Strategy: shard the 32 output rows across 8 NeuronCores (4 rows each).
Host pre-packs weights/x into fp16 DMA-friendly layouts; on each core the
per-location [K=576] contractions run as fp16 matmuls with fp32 PSUM
accumulation. K is chunked 128/64 via a kw-paired duplicated x tile
(partitions 64-127 hold x shifted one pixel), and two oh-adjacent
locations sharing the same rhs pixel pair are packed along the stationary
M dimension (M=128) to halve weight-load cost. Bias is added during the
PSUM->SBUF drain (per-partition scalar add on VectorE).
"""

import numpy as np

import concourse.bass as bass  # noqa: F401  (bass types referenced via bacc)
import concourse.mybir as mybir
import concourse.tile as tile
from concourse import bacc
from concourse.bass_utils import run_bass_kernel_spmd

# problem shape (hardcoded per contest contract)
B = 64
C = 64
H = W = 32
O = 64
OH = OW = 32
N_CORES = 8
R = 4  # oh rows per core
XW = 34  # padded width entries (-1..32)
XCOLS = 6 * XW * B  # 13056
WCOLS = 32 * 768  # 24576
OWB = 4  # ow columns per weight DMA block
F16 = mybir.dt.float16
F32 = mybir.dt.float32

_NC_CACHE = {}


def _build(n_cores=N_CORES, w_bufs=3, psum_bufs=6):
    nc = bacc.Bacc("TRN2", target_bir_lowering=False, debug=False,
                   num_devices=n_cores)

    x_d = nc.dram_tensor("xp", [64, XCOLS], F16, kind="ExternalInput")
    wf_d = nc.dram_tensor("wf", [128, WCOLS], F16, kind="ExternalInput")
    wh_d = nc.dram_tensor("wh", [64, WCOLS], F16, kind="ExternalInput")
    b_d = nc.dram_tensor("bias", [128, 64], F32, kind="ExternalInput")
    o_d = nc.dram_tensor("out", [2, 128, 2048], F32, kind="ExternalOutput")

    with tile.TileContext(nc) as tc:
        with (
            tc.tile_pool(name="xpool", bufs=1) as xpool,
            tc.tile_pool(name="cpool", bufs=1) as cpool,
            tc.tile_pool(name="opool", bufs=1) as opool,
            tc.tile_pool(name="wfpool", bufs=w_bufs) as wfpool,
            tc.tile_pool(name="whpool", bufs=w_bufs) as whpool,
            tc.tile_pool(name="pspool", bufs=psum_bufs, space="PSUM") as pspool,
        ):
            x_sb = xpool.tile([128, XCOLS], F16)
            # partitions 0-63: direct copy; 64-127: shifted one pixel (+64)
            nc.sync.dma_start(out=x_sb[0:64, :], in_=x_d[:, :])
            nc.sync.dma_start(out=x_sb[64:128, 0 : XCOLS - 64],
                              in_=x_d[:, 64:XCOLS])

            bias_sb = cpool.tile([128, 64], F32)
            nc.sync.dma_start(out=bias_sb[:], in_=b_d[:, :])

            out_sb = [
                opool.tile([128, 2048], F32, tag=f"out{p}", name=f"out_sb{p}")
                for p in (0, 1)
            ]

            for blk in range(OW // OWB):
                wf_t = wfpool.tile([128, OWB * 768], F16)
                wh_t = whpool.tile([64, OWB * 768], F16)
                c0 = blk * OWB * 768
                nc.sync.dma_start(out=wf_t[:], in_=wf_d[:, c0 : c0 + OWB * 768])
                nc.sync.dma_start(out=wh_t[:], in_=wh_d[:, c0 : c0 + OWB * 768])
                for j in range(OWB):
                    ow = blk * OWB + j
                    for p in (0, 1):
                        ps = pspool.tile([128, 64], F32)
                        base = j * 768 + p * 384
                        hA = 1 + 2 * p
                        cF = lambda h: (h * XW + ow) * B
                        cH = lambda h: (h * XW + ow + 2) * B
                        mm = nc.tensor.matmul
                        # K=128 chunks: (kh, kw in {0,1}) pairs; M=128 packs
                        # locs (oh=r0+2p, oh=r0+2p+1) sharing the rhs pixels
                        mm(ps[0:128, :], wf_t[:, base : base + 128],
                           x_sb[:, cF(hA) : cF(hA) + 64],
                           start=True, stop=False)
                        mm(ps[0:128, :], wf_t[:, base + 128 : base + 256],
                           x_sb[:, cF(hA + 1) : cF(hA + 1) + 64],
                           start=False, stop=False)
                        mm(ps[0:64, :], wf_t[:, base + 256 : base + 320],
                           x_sb[:, cF(hA - 1) : cF(hA - 1) + 64],
                           start=False, stop=False)
                        mm(ps[64:128, :], wf_t[:, base + 320 : base + 384],
                           x_sb[:, cF(hA + 2) : cF(hA + 2) + 64],
                           start=False, stop=False)
                        # K=64 chunks: kw=2 leftovers
                        mm(ps[0:64, :], wh_t[0:64, base + 256 : base + 320],
                           x_sb[0:64, cH(hA - 1) : cH(hA - 1) + 64],
                           start=False, stop=False)
                        mm(ps[64:128, :], wh_t[0:64, base + 320 : base + 384],
                           x_sb[0:64, cH(hA + 2) : cH(hA + 2) + 64],
                           start=False, stop=False)
                        mm(ps[0:128, :], wh_t[0:64, base : base + 128],
                           x_sb[0:64, cH(hA) : cH(hA) + 64],
                           start=False, stop=False)
                        mm(ps[0:128, :], wh_t[0:64, base + 128 : base + 256],
                           x_sb[0:64, cH(hA + 1) : cH(hA + 1) + 64],
                           start=False, stop=True)
                        jcol = p * 32 + ow
                        nc.vector.tensor_scalar_add(
                            out=out_sb[p][:, ow * 64 : (ow + 1) * 64],
                            in0=ps[:, :],
                            scalar1=bias_sb[:, jcol : jcol + 1],
                        )

            for p in (0, 1):
                nc.sync.dma_start(out=o_d[p], in_=out_sb[p][:])

    nc.compile()
    return nc


def get_nc():
    if "nc" not in _NC_CACHE:
        _NC_CACHE["nc"] = _build()
    return _NC_CACHE["nc"]


# ---------------- host-side layout prep ----------------

def prep_x(x):
    xt = x.transpose(1, 2, 3, 0)  # [c, h, w, b]
    xp = np.zeros((C, H + 2, W + 2, B), np.float16)
    xp[:, 1 : H + 1, 1 : W + 1, :] = xt
    return [
        np.ascontiguousarray(xp[:, R * c : R * c + 6, :, :].reshape(C, XCOLS))
        for c in range(N_CORES)
    ]


def prep_w(weight):
    wfs, whs = [], []
    for core in range(N_CORES):
        r0 = R * core
        Wc = weight[r0 : r0 + 4]  # [4, 32, O, C, KH, KW]
        # TF[ohl, ow, kh] = [(kw01, c) = 128 rows, o = 64 cols]
        TF = (Wc[:, :, :, :, :, 0:2]
              .transpose(0, 1, 4, 5, 3, 2).reshape(4, 32, 3, 128, O))
        # TH[ohl, ow, kh] = [c = 64 rows, o = 64 cols]  (kw=2)
        TH = Wc[:, :, :, :, :, 2].transpose(0, 1, 4, 3, 2)
        fulls, halves = [], []
        for p in (0, 1):
            A, Bb = 2 * p, 2 * p + 1
            FP1 = np.concatenate([TF[A, :, 1], TF[Bb, :, 0]], axis=-1)
            FP2 = np.concatenate([TF[A, :, 2], TF[Bb, :, 1]], axis=-1)
            fulls.append(np.concatenate(
                [FP1, FP2, TF[A, :, 0], TF[Bb, :, 2]], axis=-1))
            HP1 = np.concatenate([TH[A, :, 1], TH[Bb, :, 0]], axis=-1)
            HP2 = np.concatenate([TH[A, :, 2], TH[Bb, :, 1]], axis=-1)
            halves.append(np.concatenate(
                [HP1, HP2, TH[A, :, 0], TH[Bb, :, 2]], axis=-1))
        wfull = np.concatenate(fulls, axis=-1)   # [32, 128, 768]
        whalf = np.concatenate(halves, axis=-1)  # [32, 64, 768]
        wfs.append(np.ascontiguousarray(
            wfull.transpose(1, 0, 2).reshape(128, WCOLS)).astype(np.float16))
        whs.append(np.ascontiguousarray(
            whalf.transpose(1, 0, 2).reshape(64, WCOLS)).astype(np.float16))
    return wfs, whs


def prep_bias(bias):
    outs = []
    for core in range(N_CORES):
        bc = bias[:, R * core : R * core + 4, :]  # [O, 4, OW]
        b0 = np.concatenate([bc[:, 0], bc[:, 1]], axis=0)
        b1 = np.concatenate([bc[:, 2], bc[:, 3]], axis=0)
        outs.append(np.ascontiguousarray(
            np.concatenate([b0, b1], axis=1)).astype(np.float32))
    return outs


def make_in_maps(x, weight, bias):
    xs = prep_x(np.asarray(x, dtype=np.float32))
    wfs, whs = prep_w(np.asarray(weight, dtype=np.float32))
    bs = prep_bias(np.asarray(bias, dtype=np.float32))
    return [
        {"xp": xs[c], "wf": wfs[c], "wh": whs[c], "bias": bs[c]}
        for c in range(N_CORES)
    ]


def assemble_out(per_core):
    out = np.empty((B, O, OH, OW), np.float32)
    for core in range(N_CORES):
        r0 = R * core
        dev = per_core[core].reshape(2, 2, O, OW, B)  # [p, half, o, ow, b]
        for p in (0, 1):
            for half in (0, 1):
                out[:, :, r0 + 2 * p + half, :] = dev[p, half].transpose(2, 0, 1)
    return out


def kernel(x, weight, bias):
    nc = get_nc()
    in_maps = make_in_maps(x, weight, bias)
    res = run_bass_kernel_spmd(nc, in_maps, core_ids=list(range(N_CORES)))
    return assemble_out([res.results[c]["out"] for c in range(N_CORES)])



# revision 2
# speedup vs baseline: 17.4853x; 17.4853x over previous
"""Trainium2 Bass kernel for nn_Conv2dLocal (locally-connected 2d conv,
no weight sharing).

Strategy: shard the 32 output rows across 8 NeuronCores (4 rows each).
The matmul is arranged with x STATIONARY and the per-location weights
MOVING, which matters because a locally-connected conv has zero weight
reuse: with weights stationary every matmul pays a full LDWEIGHTS for
only 64 moving columns, while with x stationary one 64-column weight
load serves up to 192+192 moving weight columns.

Per core the kernel walks 33 pixel-steps j.  SBUF x is laid out as
[128, 6*34*64]: partitions 0-63 hold x[c, row l, pixel w-1], partitions
64-127 the same shifted one pixel, so the K=128 stationary S(l,j) =
x_sb[:, (l*34+j)*64 : +64] packs (kw=0 | kw=1) x 64 channels for output
column j with batch as the 64 stationary columns.  One matmul per input
row l streams the prepacked weight columns for every (oh,kh) pair using
row l (N = 64*|groups|), accumulating into the psum tile of ow=j
(columns oh*64+o, fp32).  kw=2 contributions for ow=j-1 contract K=64
against the unshifted x at pixel j.  Bias is seeded first into each
psum tile by a K=1 matmul against a ones row (start=True), so every
other matmul simply accumulates.  The ACT engine drains finished psum
tiles into out_sb as fp16; two tail DMAs write the result out.
"""

import numpy as np

import concourse.mybir as mybir
import concourse.tile as tile
from concourse import bacc
from concourse.bass_utils import run_bass_kernel_spmd

B = 64
C = 64
O = 64
OH = OW = 32
N_CORES = 8
R = 4  # oh rows per core
XW = 34  # padded width entries (-1..32)
XCOLS = 6 * XW * B  # 13056
WCOLS2 = 32 * 768  # 24576
F16 = mybir.dt.float16
F32 = mybir.dt.float32

# moving-column block offsets per input row l (64*|{oh,kh pairs using l}|)
OFF01 = [0, 64, 192, 384, 576, 704]

_NC_CACHE = {}


def build(n_iter=1, w_bufs=4, ps_bufs=8, unroll=1, blk_steps=4,
          out_dtype="f16", opool_bufs=2):
    OD = F16 if out_dtype == "f16" else F32
    nc = bacc.Bacc("TRN2", target_bir_lowering=False, debug=False,
                   num_devices=N_CORES)
    x_d = nc.dram_tensor("xp", [64, XCOLS], F16, kind="ExternalInput")
    w01_d = nc.dram_tensor("w01", [128, WCOLS2], F16, kind="ExternalInput")
    w2_d = nc.dram_tensor("w2", [128, WCOLS2 // 2], F16, kind="ExternalInput")
    b_d = nc.dram_tensor("biasp", [1, 8192], F16, kind="ExternalInput")
    o_d = nc.dram_tensor("out", [64, 8192], OD, kind="ExternalOutput")

    with tile.TileContext(nc) as tc:
        with (
            tc.tile_pool(name="xpool", bufs=1) as xpool,
            tc.tile_pool(name="cpool", bufs=1) as cpool,
            tc.tile_pool(name="opool", bufs=opool_bufs) as opool,
            tc.tile_pool(name="w01pool", bufs=w_bufs) as w01pool,
            tc.tile_pool(name="w2pool", bufs=w_bufs) as w2pool,
            tc.tile_pool(name="pspool", bufs=ps_bufs, space="PSUM") as pspool,
        ):
            x_sb = xpool.tile([128, XCOLS], F16)
            nc.sync.dma_start(out=x_sb[0:64, :], in_=x_d[:, :])
            nc.sync.dma_start(out=x_sb[64:128, 0 : XCOLS - 64],
                              in_=x_d[:, 64:XCOLS])
            bias_sb = cpool.tile([1, 8192], F16, name="bias_sb")
            nc.sync.dma_start(out=bias_sb[:], in_=b_d[:, :])
            ones_sb = cpool.tile([1, 64], F16, name="ones_sb")
            nc.vector.memset(ones_sb[:], 1.0)


            BW = blk_steps * 768
            BW2 = BW // 2

            def body():
                out_sb = opool.tile([64, 8192], OD, name="out_sb")
                w01_t = [None]
                w2_t = [None]
                ps = [None] * 32
                mm = nc.tensor.matmul
                for j in range(33):
                    if j < 32:
                        if j % blk_steps == 0:
                            w01_t[0] = w01pool.tile([128, BW], F16,
                                                    name="w01_t")
                            c0 = (j // blk_steps) * BW
                            nc.sync.dma_start(out=w01_t[0][:],
                                              in_=w01_d[:, c0 : c0 + BW])
                        wt = w01_t[0]
                        jo = (j % blk_steps) * 768
                        p = pspool.tile([64, 256], F32, name="ps")
                        ps[j] = p
                        mm(p[:, :], ones_sb[0:1, 0:64],
                           bias_sb[0:1, j * 256 : (j + 1) * 256],
                           start=True, stop=False)
                        for l in range(6):
                            g0, g1 = max(0, l - 2), min(3, l)
                            n = 64 * (g1 - g0 + 1)
                            col = (l * XW + j) * B
                            mm(p[:, g0 * 64 : (g1 + 1) * 64],
                               x_sb[:, col : col + 64],
                               wt[:, jo + OFF01[l] : jo + OFF01[l] + n],
                               start=False, stop=False)
                    if j >= 1:
                        jj = j - 1
                        if jj % blk_steps == 0:
                            w2_t[0] = w2pool.tile([128, BW2], F16,
                                                  name="w2_t")
                            c0 = (jj // blk_steps) * BW2
                            nc.sync.dma_start(out=w2_t[0][:],
                                              in_=w2_d[:, c0 : c0 + BW2])
                        wt2 = w2_t[0]
                        par = jj % 2
                        p0 = 64 * par
                        jo2 = ((jj % blk_steps) // 2) * 768
                        p = ps[jj]
                        for l in range(6):
                            g0, g1 = max(0, l - 2), min(3, l)
                            n = 64 * (g1 - g0 + 1)
                            # even ow: unshifted x at pixel jj+1 (col j+1);
                            # odd ow: shifted partitions hold pixel j at col j
                            col = (l * XW + j + 1 - par) * B
                            mm(p[:, g0 * 64 : (g1 + 1) * 64],
                               x_sb[p0 : p0 + 64, col : col + 64],
                               wt2[p0 : p0 + 64,
                                   jo2 + OFF01[l] : jo2 + OFF01[l] + n],
                               start=False, stop=(l == 5))
                        nc.scalar.copy(
                            out=out_sb[:, (j - 1) * 256 : j * 256],
                            in_=p[:, :])
                nc.sync.dma_start(out=o_d[:, 0:4096], in_=out_sb[:, 0:4096])
                nc.sync.dma_start(out=o_d[:, 4096:8192],
                                  in_=out_sb[:, 4096:8192])

            if n_iter == 1:
                for _ in range(unroll):
                    body()
            else:
                with tc.For_i(0, n_iter, 1):
                    for _ in range(unroll):
                        body()
    nc.compile()
    return nc


def get_nc():
    if "nc" not in _NC_CACHE:
        _NC_CACHE["nc"] = build(1)
    return _NC_CACHE["nc"]


# ---------------- host-side layout prep ----------------

def prep_x(x):
    xt = x.transpose(1, 2, 3, 0)  # [c, h, w, b]
    xp = np.zeros((C, 34, 34, B), np.float16)
    xp[:, 1:33, 1:33, :] = xt
    return [
        np.ascontiguousarray(xp[:, R * c : R * c + 6, :, :].reshape(C, XCOLS))
        for c in range(N_CORES)
    ]


def prep_w(weight):
    w = np.asarray(weight, np.float32)
    outs01, outs2 = [], []
    for core in range(N_CORES):
        Wc = w[R * core : R * core + 4]  # [4 g, 32 j, 64 o, 64 c, 3, 3]
        w01 = np.empty((128, 32, 768), np.float16)
        w2 = np.empty((64, 32, 768), np.float16)
        for l in range(6):
            g0, g1 = max(0, l - 2), min(3, l)
            for i, g in enumerate(range(g0, g1 + 1)):
                kh = l - g
                off = OFF01[l] + 64 * i
                blk01 = Wc[g, :, :, :, kh, 0:2]  # [j, o, c, kw]
                w01[:, :, off : off + 64] = (
                    blk01.transpose(3, 2, 0, 1).reshape(128, 32, 64))
                blk2 = Wc[g, :, :, :, kh, 2]     # [j, o, c]
                w2[:, :, off : off + 64] = blk2.transpose(2, 0, 1)
        outs01.append(np.ascontiguousarray(w01.reshape(128, WCOLS2)))
        w2p = np.zeros((128, 16, 768), np.float16)
        w2p[0:64, :, :] = w2[:, 0::2, :]
        w2p[64:128, :, :] = w2[:, 1::2, :]
        outs2.append(np.ascontiguousarray(w2p.reshape(128, WCOLS2 // 2)))
    return outs01, outs2


def prep_bias(bias):
    outs = []
    for core in range(N_CORES):
        bc = np.asarray(bias, np.float32)[:, R * core : R * core + 4, :]
        outs.append(np.ascontiguousarray(
            bc.transpose(2, 1, 0).reshape(1, 8192)).astype(np.float16))
    return outs


def make_in_maps(x, weight, bias):
    xs = prep_x(np.asarray(x, dtype=np.float32))
    w01s, w2s = prep_w(weight)
    bs = prep_bias(bias)
    return [
        {"xp": xs[c], "w01": w01s[c], "w2": w2s[c], "biasp": bs[c]}
        for c in range(N_CORES)
    ]


def assemble_out(per_core):
    out = np.empty((B, O, OH, OW), np.float32)
    for core in range(N_CORES):
        dev = per_core[core].astype(np.float32).reshape(B, 32, 4, 64)
        out[:, :, R * core : R * core + 4, :] = dev.transpose(0, 3, 2, 1)
    return out


def kernel(x, weight, bias):
    nc = get_nc()
    in_maps = make_in_maps(x, weight, bias)
    res = run_bass_kernel_spmd(nc, in_maps, core_ids=list(range(N_CORES)))
    return assemble_out([res.results[c]["out"] for c in range(N_CORES)])
